# revision 1
# baseline (speedup 1.0000x reference)
"""Trainium2 kernel for nn_LinearDynamics: chunked two-level scan, 8-core data parallel.

bf16 increment matmuls + exact-fp32 state chain (DVE adds); f32r batched
boundary-state phase; scoped PSUM pools give phase C 8-deep psum buffering.
Measured ~118us HW exec, rel err 2.7e-4 (serial fp32 baseline: 389us).

Per core (128 batch rows, state transposed xT [d_x=128, b=128]):
  x_{t+1} = x_t + (x_t @ dtA + u_t @ B2),  dtA = dt*A, B2 = dt*B
  M = I + dtA

Chunks: S=16 chunks of L=16 steps. Host precomputes (float64, cast f32):
  N_p = B2 @ M^p        p = 0..15      (phase A weights)
  MP_d = M^(d*L)        d = 0..15      (boundary-state weights)

Phase A: W_s = sum_j u_{sL+j} @ N_{L-1-j}      (PSUM-accumulated, N=512)
Phase B': X_{4g+q} = sum_d Wext @ MP_d          (batched, no serial chain;
          Wext = [0,0,0, x0, W_0..W_15] in SBUF, f32r)
Phase C: per 4-chunk group, 16 local steps:
          psum = u@B2 + x_r@dtA   (two f32r MMs, N=512)
          x_f32 <- x_f32 + psum   (exact DVE/ACT add)
          x_r   <- round(x_f32)   (cast copy, next step's MM rhs)
          DMA out x_f32

All matmuls f32r (1 cyc/row at N=512); x itself never rounded except
the one-time x0 term in phase B'.
"""

import ml_dtypes
import numpy as np

DT = 0.1
BATCH, T, DX, DU = 1024, 256, 128, 64
NCORES = 8
BPC = BATCH // NCORES  # 128
S, L = 16, 16
NG, GS = 4, 4

_CACHE = {}


def _build(debug=False, use_f32r=True):
    import concourse.mybir as mybir
    import concourse.tile as tile
    from concourse import bacc

    f32 = mybir.dt.float32
    f32r = mybir.dt.float32r if use_f32r else f32
    bf16 = mybir.dt.bfloat16
    GW = GS * BPC  # 512, group width

    nc = bacc.Bacc("TRN2", target_bir_lowering=False, debug=debug)
    w0_d = nc.declare_dram_parameter("W0T", [DX, 4 * DX], f32r, isOutput=False)
    u_d = nc.declare_dram_parameter("uT", [NG, DU, L * GW], bf16, isOutput=False)
    wt_d = nc.declare_dram_parameter("WT", [DX, (L + 1) * DX], bf16, isOutput=False)
    mp_d = nc.declare_dram_parameter("MP", [DX, L * DX], f32r, isOutput=False)
    dta_d = nc.declare_dram_parameter("DTA", [DX, DX], bf16, isOutput=False)
    y_d = nc.declare_dram_parameter("yT", [L, DX, S * BPC], f32, isOutput=True)

    with tile.TileContext(nc) as tc:
        with (
            tc.tile_pool(name="cw", bufs=1) as cw,
            tc.tile_pool(name="xc", bufs=8) as xc,
            tc.tile_pool(name="xr", bufs=8) as xrp,
        ):
            Wt = cw.tile([DX, (L + 1) * DX], bf16)
            nc.sync.dma_start(Wt[:], wt_d[:])
            MP = cw.tile([DX, L * DX], f32r)
            nc.sync.dma_start(MP[:], mp_d[:])
            dtA = cw.tile([DX, DX], bf16)
            nc.sync.dma_start(dtA[:], dta_d[:])
            # Wext: [z z z x0 | W_0..W_15] -> 20 slots
            Wext = cw.tile([DX, (4 + S) * DX], f32r)
            nc.sync.dma_start(Wext[:, 0 : 4 * DX], w0_d[:])
            X_sb = cw.tile([DX, S * BPC], f32)
            u_sb = cw.tile([DX, (L // 2) * NG * GW], bf16)

            BLK = L * GW  # 8192 columns per group block
            for g in range(NG):
                half = 64 * (g // 2)
                col0 = (g % 2) * BLK
                # two 1MB DMAs per group so phase A can start on the first half
                for h in range(2):
                    dst = u_sb[
                        half : half + 64,
                        col0 + h * (BLK // 2) : col0 + (h + 1) * (BLK // 2),
                    ]
                    nc.sync.dma_start(
                        dst, u_d[g][:, h * (BLK // 2) : (h + 1) * (BLK // 2)]
                    )

            def wslot(p, par):
                return Wt[64 * par : 64 * par + 64, p * DX : (p + 1) * DX]

            def uslice(j, g):
                par = g // 2
                off = (g % 2) * BLK + j * GW
                return u_sb[64 * par : 64 * par + 64, off : off + GW], par

            Xr_init = []

            psAX = tc.tile_pool(name="psA", bufs=2, space="PSUM")
            psA = psAX.__enter__()
            psXc = tc.tile_pool(name="psX", bufs=2, space="PSUM")
            psX = psXc.__enter__()

            def emit_A(g):
                ps = psA.tile([DX, GW], f32)
                for j in range(L):
                    p = L - 1 - j
                    rhs, par = uslice(j, g)
                    nc.tensor.matmul(
                        ps[:], wslot(p, par), rhs,
                        start=(j == 0), stop=(j == L - 1),
                    )
                # W_s slots live at (4 + s) in Wext
                nc.scalar.copy(
                    Wext[:, (4 + g * GS) * DX : (4 + (g + 1) * GS) * DX], ps[:]
                )

            def emit_Bp(g):
                ps = psX.tile([DX, GW], f32)
                nd = GS * g + 4
                for d in range(nd):
                    start_col = (GS * g + 3 - d) * DX
                    nc.tensor.matmul(
                        ps[:],
                        MP[:, d * DX : (d + 1) * DX],
                        Wext[:, start_col : start_col + GW],
                        start=(d == 0), stop=(d == nd - 1),
                    )
                nc.vector.tensor_copy(X_sb[:, g * GW : (g + 1) * GW], ps[:])
                xr0 = xrp.tile([DX, GW], bf16)
                nc.scalar.copy(xr0[:], ps[:])
                Xr_init.append(xr0)

            for g in range(NG):
                emit_A(g)
                emit_Bp(g)

            psXc.__exit__(None, None, None)
            psAX.__exit__(None, None, None)
            psCc = tc.tile_pool(name="psC", bufs=8, space="PSUM")
            psC = psCc.__enter__()

            # phase C
            xg_f32 = [X_sb[:, g * GW : (g + 1) * GW] for g in range(NG)]
            xg_r = [Xr_init[g][:] for g in range(NG)]
            for k in range(1, L + 1):
                for g in range(NG):
                    rhs_u, par = uslice(k - 1, g)
                    ps = psC.tile([DX, GW], f32)
                    nc.tensor.matmul(ps[:], wslot(L, par), rhs_u, start=True, stop=False)
                    nc.tensor.matmul(ps[:], dtA[:], xg_r[g], start=False, stop=True)
                    xnew = xc.tile([DX, GW], f32)
                    nc.vector.tensor_add(xnew[:], ps[:], xg_f32[g])
                    if k < L:
                        xrn = xrp.tile([DX, GW], bf16)
                        nc.scalar.copy(xrn[:], xnew[:])
                        xg_r[g] = xrn[:]
                    xg_f32[g] = xnew[:]
                    dst = y_d[k - 1][:, g * GW : (g + 1) * GW]
                    nc.sync.dma_start(dst, xnew[:])
            psCc.__exit__(None, None, None)
    nc.compile()
    return nc


def _get_nc():
    if "nc" not in _CACHE:
        _CACHE["nc"] = _build()
    return _CACHE["nc"]


def _host_mats(A, Bmat):
    M64 = np.eye(DX, dtype=np.float64) + DT * A.astype(np.float64)
    B264 = DT * Bmat.astype(np.float64)
    dtA = (DT * A.astype(np.float64)).astype(np.float32)
    Wt = np.zeros((DX, (L + 1) * DX), dtype=np.float32)
    Mp = np.eye(DX, dtype=np.float64)
    for p in range(L):
        Np = (B264 @ Mp).astype(np.float32)
        Wt[0:DU, p * DX : (p + 1) * DX] = Np
        Wt[DU : 2 * DU, p * DX : (p + 1) * DX] = Np
        Mp = Mp @ M64
    B2 = B264.astype(np.float32)
    Wt[0:DU, L * DX : (L + 1) * DX] = B2
    Wt[DU : 2 * DU, L * DX : (L + 1) * DX] = B2
    ML64 = Mp  # M^L
    MP = np.zeros((DX, L * DX), dtype=np.float32)
    Md = np.eye(DX, dtype=np.float64)
    for d in range(L):
        MP[:, d * DX : (d + 1) * DX] = Md.astype(np.float32)
        Md = Md @ ML64
    return dtA, Wt, MP


def _prep_inputs(initial_state, u_traj, A, Bmat):
    dtA, Wt, MP = _host_mats(A, Bmat)
    in_maps = []
    for c in range(NCORES):
        rc = slice(c * BPC, (c + 1) * BPC)
        w0 = np.zeros((DX, 4 * DX), dtype=np.float32)
        w0[:, 3 * DX :] = initial_state[rc].T
        uc = u_traj[rc]
        ut = uc.transpose(1, 2, 0)  # [t, k, b]
        ut = ut.reshape(S, L, DU, BPC)
        ut = ut.reshape(NG, GS, L, DU, BPC).transpose(0, 3, 2, 1, 4)  # [g,k,j,s,b]
        uT = (
            np.ascontiguousarray(ut)
            .reshape(NG, DU, L * GS * BPC)
            .astype(ml_dtypes.bfloat16)
        )
        in_maps.append(
            {
                "W0T": w0,
                "uT": uT,
                "WT": Wt.astype(ml_dtypes.bfloat16),
                "MP": MP,
                "DTA": dtA.astype(ml_dtypes.bfloat16),
            }
        )
    return in_maps


def _assemble(results, initial_state):
    out = np.empty((BATCH, T + 1, DX), dtype=np.float32)
    out[:, 0, :] = initial_state
    for c in range(NCORES):
        rc = slice(c * BPC, (c + 1) * BPC)
        yT = results[c]["yT"].reshape(L, DX, S, BPC)
        out[rc, 1:, :] = yT.transpose(3, 2, 0, 1).reshape(BPC, T, DX)
    return out


def run(initial_state, u_traj, A, Bmat, trace=False, **trace_kwargs):
    from concourse.bass_utils import run_bass_kernel_spmd

    nc = _get_nc()
    in_maps = _prep_inputs(initial_state, u_traj, A, Bmat)
    res = run_bass_kernel_spmd(
        nc, in_maps, list(range(NCORES)), trace=trace, **trace_kwargs
    )
    out = _assemble(res.results, initial_state)
    return out, res


def kernel(initial_state, u_traj, A, Bmat):
    out, _ = run(initial_state, u_traj, A, Bmat)
    return out



# revision 8
# speedup vs baseline: 1.4381x; 1.4381x over previous
"""Trainium2 kernel for nn_LinearDynamics: chunked two-level scan, 8-core data parallel.

v2: PSUM-resident state chain + full-width input DMA + paired phase A +
Horner phase B' + contiguous output blocks + DVE/ACT-alternating evacuation.

Per core (128 batch rows, state transposed xT [d_x=128, b=128]):
  x_{t+1} = x_t + (x_t @ dtA + u_t @ B2),  dtA = dt*A, B2 = dt*B
  M = I + dtA

Chunks: S=16 chunks of L=16 steps, processed as NB=4 col-blocks of 4
chunks (GW=512 cols = 4 chunks x 128 batch). Host precomputes (float64,
cast down):
  N_p  = B2 @ M^p          p = 0..15   (phase A conv weights, bf16,
                                        stacked in step-pairs)
  MP_d = M^(d*L)           d = 0..3    (f32, B' weights)
  M4L  = M^(4*L)                       (f32, B' Horner hop)

Phase A (per block g): W_s = sum_j u_{sL+j} @ N_{L-1-j} via 8 paired
  matmuls (two timesteps stacked into 128 contraction rows).
Phase B' (Horner): Y_g = Y_{g-1} @ M4L + sum_d W-window @ MP_d; Y_0 from
  x0 slot. Accumulated directly into block g's persistent PSUM bank.
Phase C: the persistent bank holds running x^T; per step two matmuls
  accumulate dt increments (u@B2 + x@dtA, start=False), then a pure copy
  (DVE for blocks 0/1, ACT for blocks 2/3) evacuates x_k to SBUF; that
  f32 tile is DMA'd out and bitcast to f32r as the next step's moving
  operand. No bf16 rounding of the state anywhere.
"""

import ml_dtypes
import numpy as np

DT = 0.1
BATCH, T, DX, DU = 1024, 256, 128, 64
NCORES = 8
BPC = BATCH // NCORES  # 128
S, L = 16, 16
NB, GS = 4, 4  # 4 col-blocks of 4 chunks
GW = GS * BPC  # 512

_CACHE = {}

# Phase C state-update mode: True = accumulate increments onto the
# persistent PSUM bank (pure-copy evacuation, DVE/ACT alternating);
# False = fresh bank per step + DVE tensor_add (fallback).
PSUM_RES = True


def _build(debug=False):
    import concourse.mybir as mybir
    import concourse.tile as tile
    from concourse import bacc

    f32 = mybir.dt.float32
    f32r = mybir.dt.float32r
    bf16 = mybir.dt.bfloat16

    nc = bacc.Bacc("TRN2", target_bir_lowering=False, debug=debug)
    wa_d = nc.declare_dram_parameter("WA", [DX, 8 * DX], bf16, isOutput=False)
    wb2_d = nc.declare_dram_parameter("WB2", [DX, DX], bf16, isOutput=False)
    dta_d = nc.declare_dram_parameter("DTA", [DX, DX], f32r, isOutput=False)
    mp_d = nc.declare_dram_parameter("MP", [DX, 5 * DX], f32r, isOutput=False)
    w0_d = nc.declare_dram_parameter("W0T", [DX, 4 * DX], f32r, isOutput=False)
    u_d = nc.declare_dram_parameter("uT", [NB, DX, 8 * GW], bf16, isOutput=False)
    y_d = nc.declare_dram_parameter("yT", [L, NB, DX, GW], f32r, isOutput=True)

    with tile.TileContext(nc) as tc:
        with (
            tc.tile_pool(name="cw", bufs=1) as cw,
            tc.tile_pool(name="xc", bufs=16) as xc,
            tc.tile_pool(name="psA", bufs=2, space="PSUM") as psA,
            tc.tile_pool(name="psX", bufs=1, space="PSUM") as psX,
        ):
            WA = cw.tile([DX, 8 * DX], bf16)
            nc.sync.dma_start(WA[:], wa_d[:])
            WB2 = cw.tile([DX, DX], bf16)
            nc.sync.dma_start(WB2[:], wb2_d[:])
            DTA = cw.tile([DX, DX], f32r)
            nc.sync.dma_start(DTA[:], dta_d[:])
            MP = cw.tile([DX, 5 * DX], f32r)
            nc.sync.dma_start(MP[:], mp_d[:])
            # Wext slots: [z z z x0 | W_0..W_15] (f32r), 20 slots of DX cols
            Wext = cw.tile([DX, 20 * DX], f32r)
            nc.sync.dma_start(Wext[:, 0 : 4 * DX], w0_d[:])
            u_sb = cw.tile([DX, NB * 8 * GW], bf16)
            for g in range(NB):
                nc.sync.dma_start(
                    u_sb[:, g * 8 * GW : (g + 1) * 8 * GW], u_d[g][:]
                )

            def ublk(g, m):
                # [128, GW]: pair m of block g; rows 0-63 step j=2m, 64-127 j=2m+1
                off = g * 8 * GW + m * GW
                return u_sb[:, off : off + GW]

            Xbank = [
                psX.tile([DX, GW], f32, name=f"Xbank{g}") for g in range(NB)
            ]
            xprev = [None] * NB
            ybound = [None] * NB  # B' boundary tiles (Horner carrier)
            csteps = [0] * NB

            def emit_A(g):
                ps = psA.tile([DX, GW], f32)
                for m in range(8):
                    nc.tensor.matmul(
                        ps[:], WA[:, m * DX : (m + 1) * DX], ublk(g, m),
                        start=(m == 0), stop=(m == 7),
                    )
                nc.scalar.copy(
                    Wext[:, (4 + 4 * g) * DX : (8 + 4 * g) * DX], ps[:]
                )

            def emit_Bp(g):
                psb = Xbank[g]
                if g == 0:
                    for d in range(4):
                        c0 = (3 - d) * DX
                        nc.tensor.matmul(
                            psb[:], MP[:, d * DX : (d + 1) * DX],
                            Wext[:, c0 : c0 + GW],
                            start=(d == 0), stop=(d == 3),
                        )
                else:
                    nc.tensor.matmul(
                        psb[:], MP[:, 4 * DX : 5 * DX],
                        ybound[g - 1],
                        start=True, stop=False,
                    )
                    for d in range(4):
                        c0 = (3 + 4 * g - d) * DX
                        nc.tensor.matmul(
                            psb[:], MP[:, d * DX : (d + 1) * DX],
                            Wext[:, c0 : c0 + GW],
                            start=False, stop=(d == 3),
                        )
                y0 = xc.tile([DX, GW], f32r)
                nc.vector.tensor_copy(y0[:], psb[:])
                ybound[g] = y0[:]
                xprev[g] = y0[:]

            def emit_C_step(g):
                k0 = csteps[g]
                if k0 >= L:
                    return False
                k = k0 + 1
                j = k - 1
                m, o = divmod(j, 2)
                rhs_u = ublk(g, m)[64 * o : 64 * o + 64, :]
                lhs_u = WB2[64 * o : 64 * o + 64, :]
                if PSUM_RES:
                    ps = Xbank[g]
                    nc.tensor.matmul(
                        ps[:], lhs_u, rhs_u,
                        start=False, stop=False, skip_group_check=True,
                    )
                    nc.tensor.matmul(
                        ps[:], DTA[:], xprev[g],
                        start=False, stop=True, skip_group_check=True,
                    )
                    xnew = xc.tile([DX, GW], f32r)
                    if g < 2:
                        nc.vector.tensor_copy(xnew[:], ps[:])
                    else:
                        nc.scalar.copy(xnew[:], ps[:])
                else:
                    ps = psA.tile([DX, GW], f32)
                    nc.tensor.matmul(ps[:], lhs_u, rhs_u, start=True, stop=False)
                    nc.tensor.matmul(
                        ps[:], DTA[:], xprev[g],
                        start=False, stop=True,
                    )
                    xnew = xc.tile([DX, GW], f32r)
                    nc.vector.tensor_add(xnew[:], ps[:], xprev[g])
                nc.sync.dma_start(y_d[k - 1][g][:], xnew[:])
                xprev[g] = xnew[:]
                csteps[g] = k
                return True

            for g in range(NB):
                emit_A(g)
                emit_Bp(g)
                for gg in range(g + 1):
                    emit_C_step(gg)
            while True:
                any_left = False
                for g in range(NB):
                    if emit_C_step(g):
                        any_left = True
                if not any_left:
                    break
    nc.compile()
    return nc


def _get_nc():
    if "nc" not in _CACHE:
        _CACHE["nc"] = _build()
    return _CACHE["nc"]


def _host_mats(A, Bmat):
    M64 = np.eye(DX, dtype=np.float64) + DT * A.astype(np.float64)
    B264 = DT * Bmat.astype(np.float64)
    # N_p = B2 @ M^p, p = 0..15
    Np = []
    Mp = np.eye(DX, dtype=np.float64)
    for p in range(L):
        Np.append(B264 @ Mp)
        Mp = Mp @ M64
    ML64 = Mp  # M^L
    WA = np.zeros((DX, 8 * DX), dtype=np.float32)
    for m in range(8):
        WA[0:DU, m * DX : (m + 1) * DX] = Np[L - 1 - 2 * m]
        WA[DU : 2 * DU, m * DX : (m + 1) * DX] = Np[L - 2 - 2 * m]
    WB2 = np.zeros((DX, DX), dtype=np.float32)
    WB2[0:DU] = B264
    WB2[DU : 2 * DU] = B264
    dtA = (DT * A.astype(np.float64)).astype(np.float32)
    MP = np.zeros((DX, 5 * DX), dtype=np.float32)
    Md = np.eye(DX, dtype=np.float64)
    for d in range(4):
        MP[:, d * DX : (d + 1) * DX] = Md.astype(np.float32)
        Md = Md @ ML64
    MP[:, 4 * DX : 5 * DX] = Md.astype(np.float32)  # M^(4L)
    return WA, WB2, dtA, MP


def _prep_inputs(initial_state, u_traj, A, Bmat):
    WA, WB2, dtA, MP = _host_mats(A, Bmat)
    WAb = WA.astype(ml_dtypes.bfloat16)
    WB2b = WB2.astype(ml_dtypes.bfloat16)
    # u layout: [core][g, par=o*64+du, m*GW + lane*BPC + b]
    # t = s*L + j, s = 4g + lane, j = 2m + o
    in_maps = []
    for c in range(NCORES):
        rc = slice(c * BPC, (c + 1) * BPC)
        uc = u_traj[rc]  # [b, t, du]
        ut = uc.reshape(BPC, NB, GS, 8, 2, DU)  # [b, g, lane, m, o, du]
        ut = ut.transpose(1, 4, 5, 3, 2, 0)  # [g, o, du, m, lane, b]
        uT = np.ascontiguousarray(ut).reshape(NB, DX, 8 * GW).astype(
            ml_dtypes.bfloat16
        )
        w0 = np.zeros((DX, 4 * DX), dtype=np.float32)
        w0[:, 3 * DX :] = initial_state[rc].T
        in_maps.append(
            {
                "WA": WAb,
                "WB2": WB2b,
                "DTA": dtA,
                "MP": MP,
                "W0T": w0,
                "uT": uT,
            }
        )
    return in_maps


def _assemble(results, initial_state):
    out = np.empty((BATCH, T + 1, DX), dtype=np.float32)
    out[:, 0, :] = initial_state
    for c in range(NCORES):
        rc = slice(c * BPC, (c + 1) * BPC)
        yT = results[c]["yT"].reshape(L, NB, DX, GS, BPC)  # [k, g, dx, lane, b]
        out[rc, 1:, :] = yT.transpose(4, 1, 3, 0, 2).reshape(BPC, T, DX)
    return out


def run(initial_state, u_traj, A, Bmat, trace=False, **trace_kwargs):
    from concourse.bass_utils import run_bass_kernel_spmd

    nc = _get_nc()
    in_maps = _prep_inputs(initial_state, u_traj, A, Bmat)
    res = run_bass_kernel_spmd(
        nc, in_maps, list(range(NCORES)), trace=trace, **trace_kwargs
    )
    out = _assemble(res.results, initial_state)
    return out, res


def kernel(initial_state, u_traj, A, Bmat):
    out, _ = run(initial_state, u_traj, A, Bmat)
    return out


# revision 11
# speedup vs baseline: 1.4855x; 1.0330x over previous
"""Trainium2 kernel for nn_LinearDynamics: chunked two-level scan, 8-core data parallel.

v4: fp16 datapath + PSUM-resident state chain + multi-ring input DMA +
paired phase A + Horner phase B' + fp16 output (host casts to f32).

Per core (128 batch rows, state transposed xT [d_x=128, b=128]):
  x_{t+1} = x_t + (x_t @ dtA + u_t @ B2),  dtA = dt*A, B2 = dt*B

Chunks: S=16 chunks of L=16 steps, processed as NB=4 col-blocks of 4
chunks (GW=512 cols = 4 chunks x 128 batch). Host precomputes (float64,
cast fp16):
  N_p  = B2 @ M^p          p = 0..15   (phase A conv weights, step-pair
                                        stacked into 128 contraction rows)
  MP_d = M^(d*L)           d = 0..4    (B' weights; d=4 is the Horner hop)

Phase A (per block g): W_s = sum_j u_{sL+j} @ N_{L-1-j}, 8 paired MMs.
Phase B' (Horner): Y_g = Y_{g-1} @ M^(4L) + sum_d W-window @ MP_d,
  accumulated into block g's persistent PSUM bank; DVE-copies the fp16
  boundary tile (carrier + first chain rhs).
Phase C: persistent bank holds running x^T in exact f32; per step two
  fp16 MMs accumulate dt-increments (start=False); one pure copy
  (DVE blocks 0/2, ACT blocks 1/3) rounds to fp16 — that tile is both
  the DMA source and the next step's moving operand. Outputs leave as
  fp16 (two steps per 256KB DMA); the host casts to f32.

Everything 16-bit is fp16 (10-bit mantissa), not bf16: inputs/outputs
span |x| <= ~50, well inside fp16 range, so fp16 is ~8x more precise.
"""

import ml_dtypes
import numpy as np

DT = 0.1
BATCH, T, DX, DU = 1024, 256, 128, 64
NCORES = 8
BPC = BATCH // NCORES  # 128
S, L = 16, 16
NB, GS = 4, 4  # 4 col-blocks of 4 chunks
GW = GS * BPC  # 512

_CACHE = {}


def _build(debug=False):
    import concourse.mybir as mybir
    import concourse.tile as tile
    from concourse import bacc

    f32 = mybir.dt.float32
    fp16 = mybir.dt.float16

    nc = bacc.Bacc("TRN2", target_bir_lowering=False, debug=debug)
    # weight slots: 0-7 WA pairs | 8 WB2 | 9 DTA | 10-14 MP (M^0..M^64)
    wh_d = nc.declare_dram_parameter("WH", [DX, 15 * DX], fp16, isOutput=False)
    w0_d = nc.declare_dram_parameter("W0T", [DX, 4 * DX], fp16, isOutput=False)
    u_d = nc.declare_dram_parameter("uT", [NB, DX, 8 * GW], fp16, isOutput=False)
    y_d = nc.declare_dram_parameter(
        "yT", [NB, L // 2, DX, 2 * GW], fp16, isOutput=True
    )

    with tile.TileContext(nc) as tc:
        with (
            tc.tile_pool(name="cw", bufs=1) as cw,
            tc.tile_pool(name="yb", bufs=4) as yb,
            tc.tile_pool(name="xc", bufs=8) as xc,
            tc.tile_pool(name="psA", bufs=2, space="PSUM") as psA,
            tc.tile_pool(name="psX", bufs=1, space="PSUM") as psX,
        ):
            WH = cw.tile([DX, 15 * DX], fp16)
            # Wext slots: [z z z x0 | W_0..W_15] (fp16), 20 slots of DX cols
            Wext = cw.tile([DX, 20 * DX], fp16)
            u_sb = cw.tile([DX, NB * 8 * GW], fp16)
            # input rings: sync <- u0; scalar <- WH, W0, u1; gpsimd <- u2, u3
            nc.sync.dma_start(u_sb[:, 0 : 8 * GW], u_d[0][:])
            nc.scalar.dma_start(WH[:], wh_d[:])
            nc.scalar.dma_start(Wext[:, 0 : 4 * DX], w0_d[:])
            nc.scalar.dma_start(u_sb[:, 8 * GW : 16 * GW], u_d[1][:])
            nc.gpsimd.dma_start(u_sb[:, 16 * GW : 24 * GW], u_d[2][:])
            nc.gpsimd.dma_start(u_sb[:, 24 * GW : 32 * GW], u_d[3][:])

            def wslot(i):
                return WH[:, i * DX : (i + 1) * DX]

            def ublk(g, m):
                # [128, GW]: pair m of block g; rows 0-63 step j=2m, 64-127 j=2m+1
                off = g * 8 * GW + m * GW
                return u_sb[:, off : off + GW]

            Xbank = [
                psX.tile([DX, GW], f32, name=f"Xbank{g}") for g in range(NB)
            ]
            xprev = [None] * NB
            ybound = [None] * NB  # B' boundary tiles (Horner carrier)
            xpair = [None] * NB  # current [DX, 2*GW] output pair tile
            csteps = [0] * NB

            def emit_A(g):
                ps = psA.tile([DX, GW], f32)
                for m in range(8):
                    nc.tensor.matmul(
                        ps[:], wslot(m), ublk(g, m),
                        start=(m == 0), stop=(m == 7),
                    )
                nc.scalar.copy(
                    Wext[:, (4 + 4 * g) * DX : (8 + 4 * g) * DX], ps[:]
                )

            def emit_Bp(g):
                psb = Xbank[g]
                if g == 0:
                    for d in range(4):
                        c0 = (3 - d) * DX
                        nc.tensor.matmul(
                            psb[:], wslot(10 + d), Wext[:, c0 : c0 + GW],
                            start=(d == 0), stop=(d == 3),
                        )
                else:
                    nc.tensor.matmul(
                        psb[:], wslot(14), ybound[g - 1],
                        start=True, stop=False,
                    )
                    for d in range(4):
                        c0 = (3 + 4 * g - d) * DX
                        nc.tensor.matmul(
                            psb[:], wslot(10 + d), Wext[:, c0 : c0 + GW],
                            start=False, stop=(d == 3),
                        )
                y0 = yb.tile([DX, GW], fp16)
                nc.vector.tensor_copy(y0[:], psb[:])
                ybound[g] = y0[:]
                xprev[g] = y0[:]

            def emit_C_mm(g):
                """Emit the two increment matmuls for block g's next step."""
                k = csteps[g] + 1
                j = k - 1
                m, o = divmod(j, 2)
                ps = Xbank[g]
                nc.tensor.matmul(
                    ps[:],
                    wslot(8)[64 * o : 64 * o + 64, :],
                    ublk(g, m)[64 * o : 64 * o + 64, :],
                    start=False, stop=False, skip_group_check=True,
                )
                nc.tensor.matmul(
                    ps[:], wslot(9), xprev[g],
                    start=False, stop=True, skip_group_check=True,
                )

            def emit_C_tail(g):
                """Evacuate block g's step k and DMA out completed pairs."""
                k = csteps[g] + 1
                half = (k - 1) % 2
                if half == 0:
                    xpair[g] = xc.tile([DX, 2 * GW], fp16, name="xp")
                dst = xpair[g][:, half * GW : (half + 1) * GW]
                if g % 2 == 0:
                    nc.vector.tensor_copy(dst, Xbank[g][:])
                else:
                    nc.scalar.copy(dst, Xbank[g][:])
                if half == 1:
                    kp = (k - 1) // 2
                    nc.sync.dma_start(y_d[g][kp][:], xpair[g][:])
                xprev[g] = dst
                csteps[g] = k

            def emit_C_step(g):
                if csteps[g] >= L:
                    return False
                emit_C_mm(g)
                emit_C_tail(g)
                return True

            # ramp: after B'(g), advance blocks 0..g one step each; this
            # leaves leads [4,3,2,1] -> adjacent blocks have opposite step
            # parity, so paired u-MMs land in disjoint PE row groups.
            for g in range(NB):
                emit_A(g)
                emit_Bp(g)
                for gg in range(g + 1):
                    emit_C_step(gg)
            # steady state: emit u/x MMs for pairs of blocks adjacently
            # (row-tile concurrency for the K=64 u-MMs), then evacuations.
            while any(c < L for c in csteps):
                for pair in ((0, 1), (2, 3)):
                    live = [g for g in pair if csteps[g] < L]
                    for g in live:
                        emit_C_mm(g)
                    for g in live:
                        emit_C_tail(g)
    nc.compile()
    return nc


def _get_nc():
    if "nc" not in _CACHE:
        _CACHE["nc"] = _build()
    return _CACHE["nc"]


def _host_mats(A, Bmat):
    M64 = np.eye(DX, dtype=np.float64) + DT * A.astype(np.float64)
    B264 = DT * Bmat.astype(np.float64)
    Np = []
    Mp = np.eye(DX, dtype=np.float64)
    for p in range(L):
        Np.append(B264 @ Mp)
        Mp = Mp @ M64
    ML64 = Mp  # M^L
    WH = np.zeros((DX, 15 * DX), dtype=np.float64)
    for m in range(8):
        WH[0:DU, m * DX : (m + 1) * DX] = Np[L - 1 - 2 * m]
        WH[DU : 2 * DU, m * DX : (m + 1) * DX] = Np[L - 2 - 2 * m]
    WH[0:DU, 8 * DX : 9 * DX] = B264
    WH[DU : 2 * DU, 8 * DX : 9 * DX] = B264
    WH[:, 9 * DX : 10 * DX] = DT * A.astype(np.float64)
    Md = np.eye(DX, dtype=np.float64)
    for d in range(5):
        WH[:, (10 + d) * DX : (11 + d) * DX] = Md
        Md = Md @ ML64
    return WH.astype(np.float16)


def _prep_inputs(initial_state, u_traj, A, Bmat):
    WHh = _host_mats(A, Bmat)
    in_maps = []
    for c in range(NCORES):
        rc = slice(c * BPC, (c + 1) * BPC)
        uc = u_traj[rc]  # [b, t, du]
        ut = uc.reshape(BPC, NB, GS, 8, 2, DU)  # [b, g, lane, m, o, du]
        ut = ut.transpose(1, 4, 5, 3, 2, 0)  # [g, o, du, m, lane, b]
        uT = np.ascontiguousarray(ut).reshape(NB, DX, 8 * GW).astype(np.float16)
        w0 = np.zeros((DX, 4 * DX), dtype=np.float16)
        w0[:, 3 * DX :] = initial_state[rc].T.astype(np.float16)
        in_maps.append({"WH": WHh, "W0T": w0, "uT": uT})
    return in_maps


def _assemble(results, initial_state):
    out = np.empty((BATCH, T + 1, DX), dtype=np.float32)
    out[:, 0, :] = initial_state
    for c in range(NCORES):
        rc = slice(c * BPC, (c + 1) * BPC)
        # yT: [g, kp, dx, half, lane, b] with t-1 = 64g + 16*lane + 2*kp + half
        yT = results[c]["yT"].reshape(NB, L // 2, DX, 2, GS, BPC)
        out[rc, 1:, :] = (
            yT.transpose(5, 0, 4, 1, 3, 2)
            .reshape(BPC, T, DX)
            .astype(np.float32)
        )
    return out


def run(initial_state, u_traj, A, Bmat, trace=False, **trace_kwargs):
    from concourse.bass_utils import run_bass_kernel_spmd

    nc = _get_nc()
    in_maps = _prep_inputs(initial_state, u_traj, A, Bmat)
    res = run_bass_kernel_spmd(
        nc, in_maps, list(range(NCORES)), trace=trace, **trace_kwargs
    )
    out = _assemble(res.results, initial_state)
    return out, res


def kernel(initial_state, u_traj, A, Bmat):
    out, _ = run(initial_state, u_traj, A, Bmat)
    return out


# revision 13
# speedup vs baseline: 1.5679x; 1.0554x over previous
"""Trainium2 kernel for nn_LinearDynamics: chunked two-level scan, 8-core data parallel.

v4: fp16 datapath + PSUM-resident state chain + multi-ring input DMA +
paired phase A + Horner phase B' + fp16 output (host casts to f32).

Per core (128 batch rows, state transposed xT [d_x=128, b=128]):
  x_{t+1} = x_t + (x_t @ dtA + u_t @ B2),  dtA = dt*A, B2 = dt*B

Chunks: S=16 chunks of L=16 steps, processed as NB=4 col-blocks of 4
chunks (GW=512 cols = 4 chunks x 128 batch). Host precomputes (float64,
cast fp16):
  N_p  = B2 @ M^p          p = 0..15   (phase A conv weights, step-pair
                                        stacked into 128 contraction rows)
  MP_d = M^(d*L)           d = 0..4    (B' weights; d=4 is the Horner hop)

Phase A (per block g): W_s = sum_j u_{sL+j} @ N_{L-1-j}, 8 paired MMs.
Phase B' (Horner): Y_g = Y_{g-1} @ M^(4L) + sum_d W-window @ MP_d,
  accumulated into block g's persistent PSUM bank; DVE-copies the fp16
  boundary tile (carrier + first chain rhs).
Phase C: persistent bank holds running x^T in exact f32; per step two
  fp16 MMs accumulate dt-increments (start=False); one pure copy
  (DVE blocks 0/2, ACT blocks 1/3) rounds to fp16 — that tile is both
  the DMA source and the next step's moving operand. Outputs leave as
  fp16 (two steps per 256KB DMA); the host casts to f32.

Everything 16-bit is fp16 (10-bit mantissa), not bf16: inputs/outputs
span |x| <= ~50, well inside fp16 range, so fp16 is ~8x more precise.
"""

import ml_dtypes
import numpy as np

DT = 0.1
BATCH, T, DX, DU = 1024, 256, 128, 64
NCORES = 8
BPC = BATCH // NCORES  # 128
S, L = 16, 16
NB, GS = 4, 4  # 4 col-blocks of 4 chunks
GW = GS * BPC  # 512

_CACHE = {}


def _build(debug=False):
    import concourse.mybir as mybir
    import concourse.tile as tile
    from concourse import bacc

    f32 = mybir.dt.float32
    fp16 = mybir.dt.float16

    nc = bacc.Bacc("TRN2", target_bir_lowering=False, debug=debug)
    # weight slots: 0-7 WA pairs | 8 WB2 | 9 DTA | 10-14 MP (M^0..M^64) | 15 M
    wh_d = nc.declare_dram_parameter("WH", [DX, 16 * DX], fp16, isOutput=False)
    w0_d = nc.declare_dram_parameter("W0T", [DX, 4 * DX], fp16, isOutput=False)
    u_d = nc.declare_dram_parameter("uT", [NB, DX, 8 * GW], fp16, isOutput=False)
    y_d = nc.declare_dram_parameter(
        "yT", [NB, L // 2, DX, 2 * GW], fp16, isOutput=True
    )

    with tile.TileContext(nc) as tc:
        with (
            tc.tile_pool(name="cw", bufs=1) as cw,
            tc.tile_pool(name="yb", bufs=4) as yb,
            tc.tile_pool(name="xc", bufs=8) as xc,
            tc.tile_pool(name="psA", bufs=2, space="PSUM") as psA,
            tc.tile_pool(name="psX", bufs=1, space="PSUM") as psX,
        ):
            WH = cw.tile([DX, 16 * DX], fp16)
            # Wext slots: [z z z x0 | W_0..W_15] (fp16), 20 slots of DX cols
            Wext = cw.tile([DX, 20 * DX], fp16)
            u_sb = cw.tile([DX, NB * 8 * GW], fp16)
            # input rings: sync <- u0 (4 chunks); scalar <- W0, WH, u1;
            # gpsimd <- u2, u3. Chunking lets phase A start on chunk 0.
            for h in range(4):
                nc.sync.dma_start(
                    u_sb[:, h * 2 * GW : (h + 1) * 2 * GW],
                    u_d[0][:, h * 2 * GW : (h + 1) * 2 * GW],
                )
            nc.scalar.dma_start(Wext[:, 0 : 4 * DX], w0_d[:])
            nc.scalar.dma_start(WH[:], wh_d[:])
            for h in range(2):
                nc.scalar.dma_start(
                    u_sb[:, (8 + 4 * h) * GW : (12 + 4 * h) * GW],
                    u_d[1][:, 4 * h * GW : 4 * (h + 1) * GW],
                )
            for blk in (2, 3):
                for h in range(2):
                    c0 = (8 * blk + 4 * h) * GW
                    nc.gpsimd.dma_start(
                        u_sb[:, c0 : c0 + 4 * GW],
                        u_d[blk][:, 4 * h * GW : 4 * (h + 1) * GW],
                    )

            def wslot(i):
                return WH[:, i * DX : (i + 1) * DX]

            def ublk(g, m):
                # [128, GW]: pair m of block g; rows 0-63 step j=2m, 64-127 j=2m+1
                off = g * 8 * GW + m * GW
                return u_sb[:, off : off + GW]

            Xbank = [
                psX.tile([DX, GW], f32, name=f"Xbank{g}") for g in range(NB)
            ]
            xprev = [None] * NB
            ybound = [None] * NB  # B' boundary tiles (Horner carrier)
            xpair = [None] * NB  # current [DX, 2*GW] output pair tile
            csteps = [0] * NB

            def emit_A(g):
                ps = psA.tile([DX, GW], f32)
                for m in range(8):
                    nc.tensor.matmul(
                        ps[:], wslot(m), ublk(g, m),
                        start=(m == 0), stop=(m == 7),
                    )
                nc.scalar.copy(
                    Wext[:, (4 + 4 * g) * DX : (8 + 4 * g) * DX], ps[:]
                )

            def emit_Bp(g):
                psb = Xbank[g]
                if g == 0:
                    for d in range(4):
                        c0 = (3 - d) * DX
                        nc.tensor.matmul(
                            psb[:], wslot(10 + d), Wext[:, c0 : c0 + GW],
                            start=(d == 0), stop=(d == 3),
                        )
                else:
                    nc.tensor.matmul(
                        psb[:], wslot(14), ybound[g - 1],
                        start=True, stop=False,
                    )
                    for d in range(4):
                        c0 = (3 + 4 * g - d) * DX
                        nc.tensor.matmul(
                            psb[:], wslot(10 + d), Wext[:, c0 : c0 + GW],
                            start=False, stop=(d == 3),
                        )
                y0 = yb.tile([DX, GW], fp16)
                nc.vector.tensor_copy(y0[:], psb[:])
                ybound[g] = y0[:]
                xprev[g] = y0[:]

            def emit_C_mm(g):
                """Emit the two increment matmuls for block g's next step."""
                k = csteps[g] + 1
                j = k - 1
                m, o = divmod(j, 2)
                ps = Xbank[g]
                nc.tensor.matmul(
                    ps[:],
                    wslot(8)[64 * o : 64 * o + 64, :],
                    ublk(g, m)[64 * o : 64 * o + 64, :],
                    start=False, stop=False, skip_group_check=True,
                )
                nc.tensor.matmul(
                    ps[:], wslot(9), xprev[g],
                    start=False, stop=True, skip_group_check=True,
                )

            def emit_C_tail(g):
                """Evacuate block g's step k and DMA out completed pairs."""
                k = csteps[g] + 1
                half = (k - 1) % 2
                if half == 0:
                    xpair[g] = xc.tile([DX, 2 * GW], fp16, name="xp")
                dst = xpair[g][:, half * GW : (half + 1) * GW]
                if g % 2 == 0:
                    nc.vector.tensor_copy(dst, Xbank[g][:])
                else:
                    nc.scalar.copy(dst, Xbank[g][:])
                if half == 1:
                    kp = (k - 1) // 2
                    nc.sync.dma_start(y_d[g][kp][:], xpair[g][:])
                xprev[g] = dst
                csteps[g] = k

            def emit_k16(g):
                # chunk s's k=16 output equals boundary X_{s+1}, already in
                # the B' tiles; only chunk 15 (x_256) needs real compute.
                cp = (nc.vector.tensor_copy if g % 2 == 0 else nc.scalar.copy)
                cp(xpair[g][:, GW : GW + 3 * BPC],
                   ybound[g][:, BPC : 4 * BPC])
                if g < 3:
                    cp(xpair[g][:, GW + 3 * BPC : 2 * GW],
                       ybound[g + 1][:, 0:BPC])
                else:
                    ps6 = psA.tile([DX, BPC], f32, name="ps256", bufs=1)
                    nc.tensor.matmul(
                        ps6[:], wslot(8)[64:128, :],
                        ublk(3, 7)[64:128, 3 * BPC : 4 * BPC],
                        start=True, stop=False,
                    )
                    nc.tensor.matmul(
                        ps6[:], wslot(15), xprev[3][:, 3 * BPC : 4 * BPC],
                        start=False, stop=True,
                    )
                    nc.scalar.copy(xpair[3][:, GW + 3 * BPC : 2 * GW], ps6[:])
                nc.sync.dma_start(y_d[g][L // 2 - 1][:], xpair[g][:])

            # ramp: keep pair members (0,1) and (2,3) step-aligned so their
            # u-MMs share weight slots in the steady loop.
            emit_A(0)
            emit_Bp(0)
            emit_A(1)
            emit_Bp(1)
            for g in (0, 1):
                emit_C_mm(g)
            for g in (0, 1):
                emit_C_tail(g)
            emit_A(2)
            emit_Bp(2)
            for g in (0, 1):
                emit_C_mm(g)
            for g in (0, 1):
                emit_C_tail(g)
            emit_A(3)
            emit_Bp(3)
            # steady state: u/x MMs for a pair of blocks adjacently, then
            # their evacuations; chain stops at k=15 (k=16 via emit_k16).
            while any(c < L - 1 for c in csteps):
                for pair in ((0, 1), (2, 3)):
                    live = [g for g in pair if csteps[g] < L - 1]
                    for g in live:
                        emit_C_mm(g)
                    for g in live:
                        emit_C_tail(g)
                    for g in live:
                        if csteps[g] == L - 1:
                            emit_k16(g)
    nc.compile()
    return nc


def _get_nc():
    if "nc" not in _CACHE:
        _CACHE["nc"] = _build()
    return _CACHE["nc"]


def _host_mats(A, Bmat):
    M64 = np.eye(DX, dtype=np.float64) + DT * A.astype(np.float64)
    B264 = DT * Bmat.astype(np.float64)
    Np = []
    Mp = np.eye(DX, dtype=np.float64)
    for p in range(L):
        Np.append(B264 @ Mp)
        Mp = Mp @ M64
    ML64 = Mp  # M^L
    WH = np.zeros((DX, 16 * DX), dtype=np.float64)
    for m in range(8):
        WH[0:DU, m * DX : (m + 1) * DX] = Np[L - 1 - 2 * m]
        WH[DU : 2 * DU, m * DX : (m + 1) * DX] = Np[L - 2 - 2 * m]
    WH[0:DU, 8 * DX : 9 * DX] = B264
    WH[DU : 2 * DU, 8 * DX : 9 * DX] = B264
    WH[:, 9 * DX : 10 * DX] = DT * A.astype(np.float64)
    Md = np.eye(DX, dtype=np.float64)
    for d in range(5):
        WH[:, (10 + d) * DX : (11 + d) * DX] = Md
        Md = Md @ ML64
    WH[:, 15 * DX : 16 * DX] = M64  # single-step M for the x_256 tail
    return WH.astype(np.float16)


def _prep_inputs(initial_state, u_traj, A, Bmat):
    WHh = _host_mats(A, Bmat)
    in_maps = []
    for c in range(NCORES):
        rc = slice(c * BPC, (c + 1) * BPC)
        uc = u_traj[rc]  # [b, t, du]
        ut = uc.reshape(BPC, NB, GS, 8, 2, DU)  # [b, g, lane, m, o, du]
        ut = ut.transpose(1, 4, 5, 3, 2, 0)  # [g, o, du, m, lane, b]
        uT = np.ascontiguousarray(ut).reshape(NB, DX, 8 * GW).astype(np.float16)
        w0 = np.zeros((DX, 4 * DX), dtype=np.float16)
        w0[:, 3 * DX :] = initial_state[rc].T.astype(np.float16)
        in_maps.append({"WH": WHh, "W0T": w0, "uT": uT})
    return in_maps


def _assemble(results, initial_state):
    out = np.empty((BATCH, T + 1, DX), dtype=np.float32)
    out[:, 0, :] = initial_state
    for c in range(NCORES):
        rc = slice(c * BPC, (c + 1) * BPC)
        # yT: [g, kp, dx, half, lane, b] with t-1 = 64g + 16*lane + 2*kp + half
        yT = results[c]["yT"].reshape(NB, L // 2, DX, 2, GS, BPC)
        out[rc, 1:, :] = (
            yT.transpose(5, 0, 4, 1, 3, 2)
            .reshape(BPC, T, DX)
            .astype(np.float32)
        )
    return out


def run(initial_state, u_traj, A, Bmat, trace=False, **trace_kwargs):
    from concourse.bass_utils import run_bass_kernel_spmd

    nc = _get_nc()
    in_maps = _prep_inputs(initial_state, u_traj, A, Bmat)
    res = run_bass_kernel_spmd(
        nc, in_maps, list(range(NCORES)), trace=trace, **trace_kwargs
    )
    out = _assemble(res.results, initial_state)
    return out, res


def kernel(initial_state, u_traj, A, Bmat):
    out, _ = run(initial_state, u_traj, A, Bmat)
    return out
